# revision 12
# baseline (speedup 1.0000x reference)
"""CSA4Rec encoder on 8 Trainium2 NeuronCores.

Strategy (dest-stationary row sharding):
  - 160768 (padded) node rows split across 8 cores (20096 rows each).
  - Edges partitioned by destination core. Per core, edges are grouped by
    source window (5 windows of <=32160 rows, so gather indices fit int16)
    and by duplicate-rank of the destination row, so every dma_scatter_add
    call has unique destination rows (scatter_add races on duplicates).
  - Per layer: y = 0; for each batch: dma_gather sources from the allgathered
    table window, scale by edge vals on DVE, dma_scatter_add into local y.
    Then the sign/normalized-noise perturbation on-chip, write the layer
    output shard, AllGather into the next layer's table.
  - final = mean of the 3 layer outputs; emb_cl = layer-0 output.
"""
import numpy as np

import concourse.bass as bass
import concourse.bacc as bacc
import concourse.tile as tile
from concourse import mybir
from concourse import bass_utils

# ---- problem constants (hardcoded per task contract) ----
N_USERS = 100000
N_ITEMS = 60000
N_REAL = 160000
D = 64
N_LAYERS = 3
EPS = 0.1

NCORES = 8
RPC = 20096                 # rows per core = 157 * 128
NBLK = 157                  # RPC / 128
NPAD = NCORES * RPC         # 160768
YROWS = RPC + 128           # scatter target incl. dummy row
DUMMY = YROWS - 1

WIN = 32160                 # window stride (< int16 max)
NWIN = 5
WSIZES = [WIN, WIN, WIN, WIN, NPAD - 4 * WIN]

# per-(core,window) duplicate-rank region sizes (slots, multiples of 128);
# every dma_scatter_add call covers one region slice -> unique dest rows per call
REG = [16640, 9984, 4608, 1792, 640, 256] + [128] * 10
NJ = len(REG)
REG_OFF = np.concatenate([[0], np.cumsum(REG)[:-1]]).astype(int)
REG_SUM = int(np.sum(REG))          # 35200
SLOTS = NWIN * REG_SUM              # 176000

# SWDGE call-size limits found on HW: dma_gather crashes the device above
# 1024 indices per call; dma_scatter_add is fine well beyond that. Also bound
# concurrency (gathers: 2 bufs x 2 queues; scatters: 4 accumulator chains x
# 2 queues) to keep each queue's descriptor ring within capacity.
C_GATHER = 1024                     # hard ucode cap per gather call
C_SCATTER = 896                     # 57 ring entries; 2/queue concurrent
N_ACC = 4                           # parallel scatter accumulator chains


def _window_batches():
    """Per-window gather batches: (offset, length, [(sub_off, sub_len) scatters]).

    Gather pieces are <=C_GATHER and stay inside one region for j<5 (tail
    regions are merged); scatter subs additionally never cross region
    boundaries, so each scatter call has unique destination rows."""
    gspans = []
    for j in range(5):
        off, rem = int(REG_OFF[j]), REG[j]
        while rem:
            ln = min(C_GATHER, rem)
            gspans.append((off, ln))
            off += ln
            rem -= ln
    off, rem = int(REG_OFF[5]), REG_SUM - int(REG_OFF[5])
    while rem:
        ln = min(C_GATHER, rem)
        gspans.append((off, ln))
        off += ln
        rem -= ln
    rbs = [int(x) for x in REG_OFF] + [REG_SUM]
    out = []
    for off, ln in gspans:
        subs = []
        for rj in range(NJ):
            lo, hi = max(off, rbs[rj]), min(off + ln, rbs[rj + 1])
            o2 = lo
            while o2 < hi:
                sl = min(C_SCATTER, hi - o2)
                subs.append((o2 - off, sl))
                o2 += sl
        out.append((off, ln, subs))
    return out

WBATCH = _window_batches()

_nc_cache = {}


def _build_nc():
    if "nc" in _nc_cache:
        return _nc_cache["nc"]
    f32 = mybir.dt.float32
    nc = bacc.Bacc("TRN2", target_bir_lowering=False, debug=False,
                   enable_asserts=True, num_devices=NCORES,
                   num_swdge_queues=4)

    x_sh = nc.dram_tensor("x_sh", [RPC, D], f32, kind="ExternalInput")
    noise_d = nc.dram_tensor("noise_sh", [N_LAYERS, RPC, D], f32, kind="ExternalInput")
    gidx_d = nc.dram_tensor("gidx", [128, SLOTS // 16], mybir.dt.int16, kind="ExternalInput")
    sidx_d = nc.dram_tensor("sidx", [128, SLOTS // 16], mybir.dt.int16, kind="ExternalInput")
    vals_d = nc.dram_tensor("valsg", [128, SLOTS // 128], f32, kind="ExternalInput")
    final_o = nc.dram_tensor("final_sh", [RPC, D], f32, kind="ExternalOutput")
    embcl_o = nc.dram_tensor("embcl_sh", [RPC, D], f32, kind="ExternalOutput")

    xin_b = nc.dram_tensor("xin_b", [RPC, D], f32, kind="Internal")
    xg = [nc.dram_tensor(f"xg{k}", [NPAD, D], f32, kind="Internal", addr_space="Shared")
          for k in range(N_LAYERS)]
    y_int = [nc.dram_tensor(f"y_int{k}", [RPC, D], f32, kind="Internal")
             for k in range(N_LAYERS)]
    y_acc = [nc.dram_tensor(f"y_acc{a}", [YROWS, D], f32, kind="Internal")
             for a in range(N_ACC)]

    yv = [t[: RPC, :].rearrange("(p q) d -> p q d", p=128) for t in y_acc]
    yzv = [t[:, :].rearrange("(p q) d -> p q d", p=128) for t in y_acc]
    yiv = [t[:, :].rearrange("(p q) d -> p q d", p=128) for t in y_int]
    nv = [noise_d[k].rearrange("(p q) d -> p q d", p=128) for k in range(N_LAYERS)]
    fv = final_o[:, :].rearrange("(p q) d -> p q d", p=128)

    PCH = [(0, 40), (40, 40), (80, 40), (120, 37)]   # perturb/final chunks over 157

    with tile.TileContext(nc) as tc:
        with (
            tc.tile_pool(name="const", bufs=1) as cpool,
            tc.tile_pool(name="g", bufs=2) as gpool,
            tc.tile_pool(name="pa", bufs=2) as papool,
            tc.tile_pool(name="pb", bufs=2) as pbpool,
            tc.tile_pool(name="pc", bufs=2) as pcpool,
            tc.tile_pool(name="acc", bufs=2) as accpool,
            tc.tile_pool(name="sm", bufs=4) as smpool,
        ):
            gidx_t = cpool.tile([128, SLOTS // 16], mybir.dt.int16, name="gidx_t")
            nc.sync.dma_start(out=gidx_t[:], in_=gidx_d[:])
            sidx_t = cpool.tile([128, SLOTS // 16], mybir.dt.int16, name="sidx_t")
            nc.sync.dma_start(out=sidx_t[:], in_=sidx_d[:])
            vals_t = cpool.tile([128, SLOTS // 128], f32, name="vals_t")
            nc.sync.dma_start(out=vals_t[:], in_=vals_d[:])
            zeros_t = cpool.tile([128, 40 * D], f32, name="zeros_t")
            nc.vector.memset(zeros_t[:], 0.0)

            # stage input shard and allgather the initial table
            nc.sync.dma_start(out=xin_b[:], in_=x_sh[:])
            nc.gpsimd.collective_compute(
                "AllGather", mybir.AluOpType.bypass,
                replica_groups=[list(range(NCORES))],
                ins=[xin_b[:, :].opt()], outs=[xg[0][:, :].opt()])

            g_i = 0
            s_i = 0
            for k in range(N_LAYERS):
                # zero the scatter accumulators (158 q-chunks of <=40)
                for a in range(N_ACC):
                    for q0, qn in [(0, 40), (40, 40), (80, 40), (120, 38)]:
                        nc.sync.dma_start(out=yzv[a][:, q0:q0 + qn, :],
                                          in_=zeros_t[:, : qn * D].rearrange(
                                              "p (q d) -> p q d", d=D))

                # SpMM: gather -> scale -> scatter-add
                for w in range(NWIN):
                    src = xg[k][w * WIN: w * WIN + WSIZES[w], :]
                    for off, ln, subs in WBATCH:
                        s0 = w * REG_SUM + off
                        cn = ln // 128
                        g_t = gpool.tile([128, C_GATHER // 128, D], f32, tag="g",
                                         name="g_t")
                        nc.gpsimd.dma_gather(
                            out_ap=g_t[:, :cn, :], in_ap=src,
                            idxs_ap=gidx_t[:, s0 // 16: (s0 + ln) // 16],
                            num_idxs=ln, num_idxs_reg=ln, elem_size=D,
                            queue_num=g_i % 2)
                        g_i += 1
                        nc.vector.tensor_tensor(
                            out=g_t[:, :cn, :], in0=g_t[:, :cn, :],
                            in1=vals_t[:, s0 // 128: (s0 + ln) // 128]
                                .to_broadcast([128, cn, D]),
                            op=mybir.AluOpType.mult)
                        for so, sl in subs:
                            nc.gpsimd.dma_scatter_add(
                                out_ap=y_acc[s_i % N_ACC][:, :],
                                in_ap=g_t[:, so // 128: (so + sl) // 128, :],
                                idxs_ap=sidx_t[:, (s0 + so) // 16: (s0 + so + sl) // 16],
                                num_idxs=sl, num_idxs_reg=sl, elem_size=D,
                                queue_num=2 + s_i % 2)
                            s_i += 1

                # perturbation: y += sign(y) * (noise/||noise||) * eps
                for q0, qn in PCH:
                    y_t = papool.tile([128, 40, D], f32, tag="y", name="y_t")
                    nc.sync.dma_start(out=y_t[:, :qn, :], in_=yv[0][:, q0:q0 + qn, :])
                    for a in range(1, N_ACC):
                        b_t = accpool.tile([128, 40, D], f32, tag="acc", name="b_t")
                        nc.sync.dma_start(out=b_t[:, :qn, :],
                                          in_=yv[a][:, q0:q0 + qn, :])
                        nc.vector.tensor_tensor(out=y_t[:, :qn, :],
                                                in0=y_t[:, :qn, :],
                                                in1=b_t[:, :qn, :],
                                                op=mybir.AluOpType.add)
                    n_t = pbpool.tile([128, 40, D], f32, tag="n", name="n_t")
                    nc.sync.dma_start(out=n_t[:, :qn, :], in_=nv[k][:, q0:q0 + qn, :])

                    sq_t = pcpool.tile([128, 40, D], f32, tag="t2", name="sq_t")
                    nc.vector.tensor_tensor(out=sq_t[:, :qn, :], in0=n_t[:, :qn, :],
                                            in1=n_t[:, :qn, :], op=mybir.AluOpType.mult)
                    nsq_t = smpool.tile([128, 40], f32, tag="sm", name="nsq_t")
                    nc.vector.reduce_sum(out=nsq_t[:, :qn], in_=sq_t[:, :qn, :],
                                         axis=mybir.AxisListType.X)
                    # 0.1/||n|| = 1/sqrt(100*nsq)
                    sn_t = smpool.tile([128, 40], f32, tag="sm", name="sn_t")
                    nc.scalar.activation(out=sn_t[:, :qn], in_=nsq_t[:, :qn],
                                         func=mybir.ActivationFunctionType.Sqrt,
                                         scale=100.0)
                    rn_t = smpool.tile([128, 40], f32, tag="sm", name="rn_t")
                    nc.vector.reciprocal(out=rn_t[:, :qn], in_=sn_t[:, :qn])
                    # sign(y) = (y>0) - (y<0), exact at 0
                    gt_t = pcpool.tile([128, 40, D], f32, tag="t2", name="gt_t")
                    nc.vector.tensor_scalar(out=gt_t[:, :qn, :], in0=y_t[:, :qn, :],
                                            scalar1=0.0, scalar2=None,
                                            op0=mybir.AluOpType.is_gt)
                    lt_t = pcpool.tile([128, 40, D], f32, tag="t3", name="lt_t")
                    nc.vector.tensor_scalar(out=lt_t[:, :qn, :], in0=y_t[:, :qn, :],
                                            scalar1=0.0, scalar2=None,
                                            op0=mybir.AluOpType.is_lt)
                    nc.vector.tensor_tensor(out=gt_t[:, :qn, :], in0=gt_t[:, :qn, :],
                                            in1=lt_t[:, :qn, :],
                                            op=mybir.AluOpType.subtract)
                    # nk_scaled = noise * (0.1/||n||); then * sign; then add
                    nc.vector.tensor_tensor(out=n_t[:, :qn, :], in0=n_t[:, :qn, :],
                                            in1=rn_t[:, :qn].to_broadcast([128, qn, D]),
                                            op=mybir.AluOpType.mult)
                    nc.vector.tensor_tensor(out=n_t[:, :qn, :], in0=n_t[:, :qn, :],
                                            in1=gt_t[:, :qn, :], op=mybir.AluOpType.mult)
                    nc.vector.tensor_tensor(out=y_t[:, :qn, :], in0=y_t[:, :qn, :],
                                            in1=n_t[:, :qn, :], op=mybir.AluOpType.add)
                    nc.sync.dma_start(out=yiv[k][:, q0:q0 + qn, :], in_=y_t[:, :qn, :])

                if k + 1 < N_LAYERS:
                    nc.gpsimd.collective_compute(
                        "AllGather", mybir.AluOpType.bypass,
                        replica_groups=[list(range(NCORES))],
                        ins=[y_int[k][:, :].opt()], outs=[xg[k + 1][:, :].opt()])

            # final = mean of layer outputs; emb_cl = layer-0 output
            nc.sync.dma_start(out=embcl_o[:], in_=y_int[0][:])
            for q0, qn in PCH:
                a_t = papool.tile([128, 40, D], f32, tag="y", name="a_t")
                nc.sync.dma_start(out=a_t[:, :qn, :], in_=yiv[0][:, q0:q0 + qn, :])
                b_t = pbpool.tile([128, 40, D], f32, tag="n", name="b_t")
                nc.sync.dma_start(out=b_t[:, :qn, :], in_=yiv[1][:, q0:q0 + qn, :])
                nc.vector.tensor_tensor(out=a_t[:, :qn, :], in0=a_t[:, :qn, :],
                                        in1=b_t[:, :qn, :], op=mybir.AluOpType.add)
                c_t = pbpool.tile([128, 40, D], f32, tag="n", name="c_t")
                nc.sync.dma_start(out=c_t[:, :qn, :], in_=yiv[2][:, q0:q0 + qn, :])
                nc.vector.tensor_tensor(out=a_t[:, :qn, :], in0=a_t[:, :qn, :],
                                        in1=c_t[:, :qn, :], op=mybir.AluOpType.add)
                nc.vector.tensor_scalar(out=a_t[:, :qn, :], in0=a_t[:, :qn, :],
                                        scalar1=1.0 / 3.0, scalar2=None,
                                        op0=mybir.AluOpType.mult)
                nc.sync.dma_start(out=fv[:, q0:q0 + qn, :], in_=a_t[:, :qn, :])

    nc.finalize()
    _nc_cache["nc"] = nc
    return nc


def _wrap16(a):
    """[S] -> [128, S//16]: slot s at (s%16, s//16), replicated to all 8 Q7 groups."""
    t = a.reshape(-1, 16).T.copy()
    return np.tile(t, (8, 1))


def _pack_core(r, cl, v):
    """Slot arrays for one core. r: local dest rows, cl: global source cols."""
    g_slot = np.zeros(SLOTS, np.int16)
    s_slot = np.full(SLOTS, DUMMY, np.int16)
    v_slot = np.zeros(SLOTS, np.float32)
    wof = cl // WIN
    for w in range(NWIN):
        m = wof == w
        rw = r[m]
        gw = (cl[m] - w * WIN).astype(np.int16)
        vw = v[m]
        o = np.argsort(rw, kind="stable")
        rw, gw, vw = rw[o], gw[o], vw[o]
        n = len(rw)
        if n == 0:
            continue
        first = np.ones(n, bool)
        first[1:] = rw[1:] != rw[:-1]
        startpos = np.maximum.accumulate(np.where(first, np.arange(n), 0))
        rank = np.arange(n) - startpos
        base = w * REG_SUM
        used = [0] * NJ
        overflow = []
        maxrank = int(rank.max())
        for j in range(maxrank + 1):
            sel = np.where(rank == j)[0]
            if j < NJ:
                take = min(len(sel), REG[j])
                sl = slice(base + REG_OFF[j], base + REG_OFF[j] + take)
                g_slot[sl] = gw[sel[:take]]
                s_slot[sl] = rw[sel[:take]]
                v_slot[sl] = vw[sel[:take]]
                used[j] = take
                overflow.extend(sel[take:].tolist())
            else:
                overflow.extend(sel.tolist())
        if overflow:
            rowsets = [set(s_slot[base + REG_OFF[j]: base + REG_OFF[j] + used[j]].tolist())
                       for j in range(NJ)]
            for e in overflow:
                for j in range(NJ - 1, -1, -1):
                    if used[j] < REG[j] and int(rw[e]) not in rowsets[j]:
                        pos = base + REG_OFF[j] + used[j]
                        g_slot[pos] = gw[e]
                        s_slot[pos] = rw[e]
                        v_slot[pos] = vw[e]
                        used[j] += 1
                        rowsets[j].add(int(rw[e]))
                        break
                else:
                    raise RuntimeError("scatter region overflow; enlarge REG")
    return g_slot, s_slot, v_slot


def _make_in_maps(user_emb, item_emb, adj_vals, noise, adj_rows, adj_cols):
    user_emb = np.asarray(user_emb, np.float32)
    item_emb = np.asarray(item_emb, np.float32)
    adj_vals = np.asarray(adj_vals, np.float32)
    noise = np.asarray(noise, np.float32)
    adj_rows = np.asarray(adj_rows, np.int32)
    adj_cols = np.asarray(adj_cols, np.int32)

    ego = np.concatenate([user_emb, item_emb], axis=0)
    x_pad = np.zeros((NPAD, D), np.float32)
    x_pad[:N_REAL] = ego
    noise_pad = np.ones((N_LAYERS, NPAD, D), np.float32)
    noise_pad[:, :N_REAL] = noise

    # combine exact-duplicate (row, col) edges (sum their vals)
    keys = adj_rows.astype(np.int64) * N_REAL + adj_cols.astype(np.int64)
    uk, inv = np.unique(keys, return_inverse=True)
    if len(uk) != len(keys):
        adj_vals = np.bincount(inv, weights=adj_vals).astype(np.float32)
        adj_rows = (uk // N_REAL).astype(np.int32)
        adj_cols = (uk % N_REAL).astype(np.int32)

    core_of = adj_rows // RPC
    in_maps = []
    for c in range(NCORES):
        m = core_of == c
        g_slot, s_slot, v_slot = _pack_core(
            (adj_rows[m] - c * RPC).astype(np.int64),
            adj_cols[m].astype(np.int64),
            adj_vals[m])
        in_maps.append({
            "x_sh": x_pad[c * RPC: (c + 1) * RPC],
            "noise_sh": np.ascontiguousarray(noise_pad[:, c * RPC: (c + 1) * RPC]),
            "gidx": _wrap16(g_slot),
            "sidx": _wrap16(s_slot),
            "valsg": v_slot.reshape(-1, 128).T.copy(),
        })
    return in_maps


def kernel(user_emb, item_emb, adj_vals, noise, adj_rows, adj_cols):
    in_maps = _make_in_maps(user_emb, item_emb, adj_vals, noise,
                            adj_rows, adj_cols)
    nc = _build_nc()
    res = bass_utils.run_bass_kernel_spmd(nc, in_maps, core_ids=list(range(NCORES)))

    final = np.concatenate([res.results[c]["final_sh"] for c in range(NCORES)])[:N_REAL]
    embcl = np.concatenate([res.results[c]["embcl_sh"] for c in range(NCORES)])[:N_REAL]
    return (final[:N_USERS], final[N_USERS:],
            embcl[:N_USERS], embcl[N_USERS:])
